# revision 1
# baseline (speedup 1.0000x reference)
"""CoefficientMaxPool Trainium2 kernel (8-core data-parallel), v2.

Problem: x [32, 512, 16, 128] f32.  Irreps group into degree blocks
l=0:[0,1), l=1:[1,4), l=2:[4,9), l=3:[9,16).  Per (batch, l, channel):
find the neighbor n* maximizing the degree-block squared norm, output
that neighbor's block components -> out [32, 16, 128].

Per core (4 batches), per batch, layout X [p=128(n%128), a=4, i=16, c=128]:
  ACT : X2 = X*X (2 halves)
  DVE : block norms accumulated in place into X2 slots i=1/4/9
        (contiguous pairwise adds; strided tensor_reduce is ~1.6x slower)
  DVE : M1[p,l,c] = max over a (2-level tree)
  PE  : TM[c,l,p] = transpose(M1) (4x 128x128)
  DVE : mx[c,l] = max over p
  PE  : mxT[l,c] = transpose(mx); ACT copy -> SBUF
  PE  : GM[p,l,c] = E4_l^T @ mxT (K=4 matmuls: global max bcast to all p)
  DVE : mask[p,a,l,c] = is_equal(norm, GM bcast) -- exact fp32 compare,
        unique winner; bf16 mask (0/1 exact)
  DVE : Xs = X * mask[l(i)] in bf16 (output rounded ~2^-8, rel err ~3e-3)
  PE  : out[1, i*c] += ones^T @ Xs (bf16 moving operand, PSUM acc over a)
  ACT : PSUM -> SBUF, DMA out.

History: baseline (PE-transpose-norms + fp32 finals) 146.4us ->
a-max-first + GM-replicate + bf16 select/finals 137.6us ->
software-pipelined stages + per-half norm adds 125.3-128.9us
(run-to-run variance ~3us).  DVE-bound (~91us busy, window nearly
gap-free); remaining overhead is ~13us fill + ~15us drain around the
4-batch pipeline.  DMA floor for 16.8MB/core is ~47us.

Hard constraints learned (do not re-derive):
- gpsimd/Pool rejects TensorTensor/TensorScalar at walrus codegen
  ("engine check failed") and cannot access PSUM; only memset/custom
  ucode ops run there.  All elementwise work must go to DVE (0.96GHz,
  1 elem/cyc/lane fp32; 2x modes need 2-byte dtypes, NOT all-SBUF).
- DMA cannot read PSUM; only ACT/DVE copy PSUM->SBUF.  Only gpsimd
  DMAs can cast dtypes or accum, but SBUF<->SBUF DMA traffic competes
  with the 47us HBM floor.
- fp32r matmul (1 cyc/row if moving>=256) requires EVERY writer of the
  operand's memory location to emit fp32r-rounded output (verifier is
  location-granular) -- hence the separate bf16 Xs tile instead.
- PE operand base partitions must be 0/32/64/96; fp32/fp32r weights
  cannot use standalone ldweights (matmul() self-loads).
- Strided DVE reads (e.g. tensor_reduce over stride-128 i) run ~1.6x
  slower than contiguous pairwise adds.
- Engine streams execute in emission order: software-pipeline stage
  emission (stage1(b+1) before stage2(b)) to fill cross-engine
  latency; Tile dep tracking is AP-region-granular.
- Per-quarter (per-a) DMA/square/norm splitting regressed (+2.7us):
  per-op bubbles (~105ns DVE) outweigh finer overlap below half-batch
  granularity.
- bf16 anywhere in the norm/argmax path flips winners (ties within
  2^-8) and fails; norms must be exact fp32 end-to-end.  bf16 is safe
  for mask (exact 0/1) and gathered outputs (rel err ~3e-3 < 2e-2).
- Winner-select as PE matmul is impossible: per-channel selection is a
  diagonal extraction (out[i,c] needs sum_n mask[n,c]*X[n,i,c], c on
  both operands) -- not expressible as a systolic contraction.

Next ideas if resumed: cut the 8.2K-PPE select pass (only remaining
big DVE item); shrink drain by interleaving last batch's per-half
selects with its PE reduce; fill is bounded by batch 0's 11.8us DMA.
"""

import os
import sys

import numpy as np

for _p in ("/opt/trn_rl_repo", "/opt/pypackages"):
    if _p not in sys.path:
        sys.path.append(_p)

from contextlib import ExitStack

import concourse.bacc as bacc
import concourse.bass as bass
import concourse.tile as tile
from concourse import mybir

N_CORES = 8
B_FULL, N, IRR, C = 32, 512, 16, 128
B = B_FULL // N_CORES  # 4 batches per core
P = 128                # partitions (n within chunk)
A = N // P             # 4 neighbor chunks
F32 = mybir.dt.float32
F32R = mybir.dt.float32r
BF16 = mybir.dt.bfloat16
ADD = mybir.AluOpType.add
MAX = mybir.AluOpType.max
MULT = mybir.AluOpType.mult
EQ = mybir.AluOpType.is_equal

_cache = {}


def _build_bass():
    nc = bacc.Bacc("TRN2", target_bir_lowering=False, debug=False,
                   num_devices=N_CORES)
    x_in = nc.dram_tensor("x", [B, N, IRR, C], F32, kind="ExternalInput")
    out_t = nc.dram_tensor("out", [B, IRR, C], F32, kind="ExternalOutput")
    ident_d = nc.inline_tensor(np.eye(P, dtype=np.float32), name="ident")
    e4_np = np.zeros((4, 4, P), dtype=np.float32)
    for l in range(4):
        e4_np[l, l, :] = 1.0
    e4_d = nc.inline_tensor(e4_np.reshape(4, 4 * P), name="e4")

    with tile.TileContext(nc) as tc, ExitStack() as ctx:
        # DRAM view: n = a*P + p  ->  [b, p, a, i, c]
        x_v = x_in.ap().rearrange("b (a p) i c -> b p a i c", p=P)
        out_v = out_t.ap().rearrange("b i c -> (b i c)").unsqueeze(0)

        xp = ctx.enter_context(tc.tile_pool(name="xp", bufs=2))
        x2p = ctx.enter_context(tc.tile_pool(name="x2p", bufs=2))
        med = ctx.enter_context(tc.tile_pool(name="med", bufs=3))
        xsp = ctx.enter_context(tc.tile_pool(name="xsp", bufs=2))
        obp = ctx.enter_context(tc.tile_pool(name="obp", bufs=1))
        singles = ctx.enter_context(tc.tile_pool(name="singles", bufs=1))
        # PSUM: TM bufs=1 (1 bank) + GM (2) + mxT (1) + pout 2x[1,2,512] (4)
        tmp_ps = ctx.enter_context(tc.tile_pool(name="tmp_ps", bufs=1,
                                                space="PSUM"))
        gm_ps = ctx.enter_context(tc.tile_pool(name="gm_ps", bufs=2,
                                               space="PSUM"))
        mxt_ps = ctx.enter_context(tc.tile_pool(name="mxt_ps", bufs=1,
                                                space="PSUM"))
        pout = ctx.enter_context(tc.tile_pool(name="pout", bufs=2,
                                              space="PSUM"))

        ones = singles.tile([P, 1], BF16)
        nc.vector.memset(ones, 1.0)
        # Prewarm the ACT Square table (~1.3us) before real data arrives
        warm = singles.tile([P, 1], F32)
        nc.vector.memset(warm, 0.0)
        nc.scalar.activation(warm, warm, mybir.ActivationFunctionType.Square)
        # E4[:, l, :] is the [4, 128] stationary that replicates row l of a
        # [4, *] moving operand to all 128 output partitions.  Constants go
        # on the ACT HWDGE ring so they don't delay batch 0's input DMAs
        # on the sync ring.
        E4 = singles.tile([4, 4, P], F32)
        nc.scalar.dma_start(out=E4.rearrange("p l j -> p (l j)"),
                            in_=e4_d.ap())
        ident = singles.tile([P, P], F32)
        nc.scalar.dma_start(out=ident, in_=ident_d.ap())

        def stage1(b):
            """DMA in, squares, norms, a-max, transpose-max, GM broadcast."""
            X = xp.tile([P, A, IRR, C], F32, tag="X")
            X2 = x2p.tile([P, A, IRR, C], F32, tag="X2")
            for h in range(2):
                ha = slice(2 * h, 2 * h + 2)
                nc.sync.dma_start(out=X[:, ha], in_=x_v[b][:, ha])
                nc.scalar.activation(X2[:, ha], X[:, ha],
                                     mybir.ActivationFunctionType.Square)

            # block norms for l=1,2,3 accumulated in place into X2 slots
            # 1/4/9 (contiguous pairwise adds; stride-128 tensor_reduce over
            # i is far slower on DVE).  Split per DMA half so the a01 adds
            # start right after square-h0, during the h1 DMA.  [DVE]
            for ha in (slice(0, 2), slice(2, 4)):
                for j, (st, e) in enumerate(((1, 4), (4, 9), (9, 16))):
                    nj = X2[:, ha, st:st + 1, :]
                    for i in range(st + 1, e):
                        nc.vector.tensor_tensor(nj, nj,
                                                X2[:, ha, i:i + 1, :], ADD)

            # M1[p, l, c] = max over a  [DVE, 2-level tree]
            M1 = med.tile([P, 4, C], F32, tag="M1")
            mt = med.tile([P, 4, C], F32, tag="mt")
            nc.vector.tensor_tensor(M1[:, 0:1, :], X2[:, 0, 0:1, :],
                                    X2[:, 1, 0:1, :], MAX)
            nc.vector.tensor_tensor(mt[:, 0:1, :], X2[:, 2, 0:1, :],
                                    X2[:, 3, 0:1, :], MAX)
            for dst, a0 in ((M1, 0), (mt, 2)):
                for j, si in enumerate((1, 4, 9)):
                    nc.vector.tensor_tensor(
                        dst[:, j + 1:j + 2, :], X2[:, a0, si:si + 1, :],
                        X2[:, a0 + 1, si:si + 1, :], MAX)
            nc.vector.tensor_tensor(M1, M1, mt, MAX)

            # TM[c, l, p] = transpose(M1)  [PE]
            TM = tmp_ps.tile([P, 4, P], F32, tag="TM")
            for l in range(4):
                nc.tensor.transpose(TM[:, l, :], M1[:, l, :], ident)

            # mx[c, l] = max over p  [DVE]
            mx = med.tile([P, 4], F32, tag="mx")
            nc.vector.tensor_reduce(out=mx, in_=TM,
                                    axis=mybir.AxisListType.X, op=MAX)

            # mxT[l, c] -> SBUF; GM[p, l, c] = bcast of global max  [PE/ACT]
            mxT = mxt_ps.tile([4, P], F32, tag="mxT")
            nc.tensor.transpose(mxT, mx, ident)
            mxs = med.tile([4, P], F32, tag="mxs")
            nc.scalar.copy(out=mxs, in_=mxT)
            GM = gm_ps.tile([P, 4, C], F32, tag="GM")
            for l in range(4):
                nc.tensor.matmul(GM[:, l, :], E4[:, l, :], mxs,
                                 start=True, stop=True)
            return X, X2, GM

        def stage2(b, X, X2, GM):
            """Mask, winner-select, PE reduce, store."""
            # mask[p, a, l, c] = (norm == global max)  [DVE, reads GM PSUM]
            mask = med.tile([P, A, 4, C], BF16, tag="mask")
            for j, si in enumerate((0, 1, 4, 9)):
                nc.vector.tensor_tensor(
                    mask[:, :, j, :], X2[:, :, si, :],
                    GM[:, j, :].unsqueeze(1).broadcast_to([P, A, C]), EQ)

            # winner-select: Xs = X * mask[l(i)] in bf16 for the PE
            # reduce (mask is exact 0/1; values rounded ~2^-8)  [DVE]
            Xs = xsp.tile([P, A, IRR, C], BF16, tag="Xs")

            def sel(s, e, l):
                nc.vector.tensor_tensor(
                    Xs[:, :, s:e, :], X[:, :, s:e, :],
                    mask[:, :, l:l + 1, :].broadcast_to([P, A, e - s, C]),
                    MULT)

            # emit in output-chunk order so the PE reduce can start on
            # chunk 0 while later selects still run
            sel(0, 1, 0)
            sel(1, 4, 1)
            sel(4, 9, 2)
            sel(9, 16, 3)

            # sum over n: bf16 PE reduce, PSUM-accumulate over a  [PE]
            Xf = Xs.rearrange("p a i c -> p a (i c)")
            ob = obp.tile([1, IRR * C], F32, tag="ob")
            for h in range(2):
                ps = pout.tile([1, 2, 512], F32, tag="ps")
                for kk in range(2):
                    k = h * 2 + kk
                    for a in range(A):
                        nc.tensor.matmul(
                            ps[:, kk, :],
                            ones,
                            Xf[:, a, k * 512:(k + 1) * 512],
                            start=(a == 0),
                            stop=(a == A - 1),
                        )
                nc.scalar.copy(out=ob[:, h * 1024:(h + 1) * 1024],
                               in_=ps.rearrange("m k f -> m (k f)"))
            nc.sync.dma_start(out=out_v[:, b * IRR * C:(b + 1) * IRR * C],
                              in_=ob)

        # Software pipeline: emit stage1(b+1) before stage2(b) so the
        # in-order DVE stream runs batch b+1's norms during batch b's
        # transpose->broadcast latency.
        live = {0: stage1(0)}
        for b in range(B):
            if b + 1 < B:
                live[b + 1] = stage1(b + 1)
            stage2(b, *live.pop(b))

    nc.compile()
    return nc


def kernel(x: np.ndarray, i2l: np.ndarray | None = None) -> np.ndarray:
    x = np.ascontiguousarray(np.asarray(x), dtype=np.float32)
    assert x.shape == (B_FULL, N, IRR, C), x.shape

    if "nc" not in _cache:
        _cache["nc"] = _build_bass()
    nc = _cache["nc"]

    from concourse.bass_utils import run_bass_kernel_spmd

    in_maps = [{"x": x[i * B:(i + 1) * B]} for i in range(N_CORES)]
    res = run_bass_kernel_spmd(nc, in_maps, list(range(N_CORES)))
    out = np.concatenate([res.results[i]["out"] for i in range(N_CORES)], axis=0)
    return out


if __name__ == "__main__":
    xs = np.random.randn(B_FULL, N, IRR, C).astype(np.float32)
    o = kernel(xs)
    print("out", o.shape, o.dtype)

